# Initial kernel scaffold
#
"""Trainium2 Bass kernel for batched per-sample expert matmul (MoE routing).

Computes y[n, i] = relu(b[idxs[n], i] + sum_o w[idxs[n], i, o] * x[n, o])
for x (8192, 256), idxs (8192,), w (64, 256, 256), b (64, 256).

Strategy
--------
Host side (numpy, cheap):
  * Stable-sort all 8192 samples by expert id, RLE the sorted ids into
    single-expert slots of <= 128 samples, and deal the slots to the 8
    cores in contiguous blocks of nseg = ceil(nslots/8) (SPMD: one
    program, per-core data).  Per-slot gathers of the expert weight
    block / bias row and a transposed x layout put the contraction dim
    on partitions.
  * All matmul operands are cast to fp16: same 1 elem/cycle PE stream
    rate as fp32r, half the HBM traffic (this kernel is DMA-bound), and
    absmax-relative error ~4e-4, well under the 2e-2 gate.

Device side (one static Tile program, identical on all 8 cores):
  for each psum group (1-2 slots sharing one PSUM bank):
      psum[m, i]  = 1 * b[i]                       (K=1 bias matmul)
      psum[m, i] += sum_p xT[p, m] * w[p, i]       (2 K-chunks per slot)
      y[m, i]     = relu(psum[m, i])               (DVE, PSUM -> fp16 SBUF)

  DMA plan: the x+w data of each slot group is packed into ONE dram
  region and streamed as one DMA per group on the single SWDGE queue
  (FIFO delivery in issue order, one completion sem per group; splitting
  across HWDGE queues makes the SDMA engines round-robin bandwidth
  between queues, which delays the earliest transfers).  Small head
  groups start compute early; small tail groups keep the final pair's
  gating transfer short.  ones+bias ride the otherwise-idle sync ring
  in one tiny DMA (Tile hoists the bias matmuls to run as soon as it
  lands, well before the bulk stream).  y output groups [2, 4, 4, tail]
  go out on sync as their relus complete.

  (The PE on these cores is pinned at 1.2GHz — HAM never opens even
  after ~4us of sustained full-width matmuls, so there is no warm-up
  train; matmul cost is 213ns per N=256.)

Host side: scatter slot rows back to the original sample order.
Pathological expert skew runs the same program over multiple passes.
"""

import os

import numpy as np

import concourse.bacc as bacc
import concourse.bass as bass
import concourse.mybir as mybir
import concourse.tile as tile
from concourse.bass_utils import run_bass_kernel_spmd

N_CORES = 8
P = 128          # SBUF/PSUM partitions
F = 256          # feature dim (in_features == out_features == 256)
SEG = 128        # samples per slot (== max PSUM partition dim)
MAX_NSEG = 32    # per-pass slot budget (SBUF residency bound)

_MM_DT_TAB = {
    "float32": mybir.dt.float32,
    "float32r": mybir.dt.float32r,
    "bfloat16": mybir.dt.bfloat16,
    "float16": mybir.dt.float16,
}
MM_DT = _MM_DT_TAB[os.environ.get("KBENCH_MM_DT", "float16")]
if MM_DT == mybir.dt.bfloat16:
    import ml_dtypes

    MM_NP: type = ml_dtypes.bfloat16
elif MM_DT == mybir.dt.float16:
    MM_NP = np.float16
else:
    MM_NP = np.float32
OUT_DT = mybir.dt.float16  # PSUM fp32 -> fp16 on the DVE relu; halves out DMA
OUT_NP = np.float16

# Set by the last kernel() call when KBENCH_TRACE=1 (used by test.py only).
LAST_EXEC_TIME_NS = None
LAST_TRACE = None


def _batches(n, sizes, rest):
    """Split range(n) into batches: explicit `sizes` first, then `rest`-sized."""
    out = []
    lo = 0
    i = 0
    while lo < n:
        sz = sizes[i] if i < len(sizes) else rest
        i += 1
        hi = min(n, lo + sz)
        out.append((lo, hi))
        lo = hi
    return out


def _groups(nseg):
    """x+w stream groups, aligned 1:1 with the psum pair groups: every
    pair is unlocked by exactly one completion sem, and the single-slot
    head/tail keep the first compute early and the final gating transfer
    short."""
    return _batches(nseg, [1], 2)


def _build_schedule(idxs: np.ndarray):
    """Sort samples by expert, cut global <=128-sample single-expert slots,
    deal them to cores in contiguous blocks of equal length."""
    B = idxs.shape[0]
    order = np.argsort(idxs, kind="stable")
    sidx = idxs[order]
    slots = []  # (expert, global_start, count), count <= SEG
    i = 0
    while i < B:
        j = i
        while j < B and sidx[j] == sidx[i]:
            j += 1
        k = i
        while k < j:
            cnt = min(SEG, j - k)
            slots.append((int(sidx[i]), k, cnt))
            k += cnt
        i = j
    nseg_pc = (len(slots) + N_CORES - 1) // N_CORES
    per_core = [
        slots[c * nseg_pc:(c + 1) * nseg_pc] for c in range(N_CORES)
    ]
    return order, per_core


def _build_program(nseg: int):
    nc = bacc.Bacc(
        "TRN2", target_bir_lowering=False, debug=False, num_devices=N_CORES
    )
    npad = nseg * SEG
    b_d = nc.dram_tensor(
        "bconst", [1, P + nseg * F], MM_DT, kind="ExternalInput"
    ).ap()
    y_d = nc.dram_tensor(
        "y", [P, nseg, F], OUT_DT, kind="ExternalOutput"
    ).ap()

    f32 = mybir.dt.float32

    # x and w stream interleaved on the single SWDGE queue in matched
    # batches (FIFO delivery in issue order at full line rate, one
    # completion sem per batch -> just-in-time pipeline; independent HWDGE
    # queues would round-robin SDMA bandwidth and delay the earliest
    # transfers).  ones/bconst ride the otherwise-idle sync ring; they are
    # only needed by the group-closing bias matmuls, off the critical path.
    wbat = _groups(nseg)
    obat = _batches(nseg, [1, 4, 4], 4)
    pairs = _batches(nseg, [1], 2)
    # packed per-group x+w layout: group g holds x (2*gsz*SEG) then w
    # (gsz*2F) columns per partition — one DMA, one completion sem per group
    goff = []
    off = 0
    for lo, hi in wbat:
        goff.append(off)
        off += (hi - lo) * (2 * SEG + 2 * F)
    XWTOT = off
    xw_d = nc.dram_tensor("xw", [P, XWTOT], MM_DT, kind="ExternalInput").ap()

    # One PSUM bank per pair when they fit (8 banks): no bank recycling
    # dependencies, and Tile hoists every bias matmul into the idle window
    # before the first stream group lands.
    ps_bufs = min(8, len(pairs))

    with tile.TileContext(nc) as tc:
        with (
            tc.tile_pool(name="const", bufs=1) as const,
            tc.tile_pool(name="w", bufs=1) as wpool,
            tc.tile_pool(name="yout", bufs=1) as ypool,
            tc.tile_pool(name="ps", bufs=ps_bufs, space="PSUM") as pspool,
        ):
            # ones+bias in one tiny sync-ring DMA; only the (freely
            # Tile-hoisted) bias matmuls need it.
            bc = const.tile([1, P + nseg * F], MM_DT, tag="bconst")
            nc.sync.dma_start(bc[:], b_d[:])
            ob = bc[:, 0:P]

            gts = {}
            for g, (lo, hi) in enumerate(wbat):
                gsz = hi - lo
                t = wpool.tile([P, gsz * (2 * SEG + 2 * F)], MM_DT,
                               tag=f"xw{g}", name=f"xw{g}")
                gts[g] = t
                nc.gpsimd.dma_start(
                    t[:], xw_d[:, goff[g]:goff[g] + gsz * (2 * SEG + 2 * F)]
                )

            seg2g = {}
            for g, (lo, hi) in enumerate(wbat):
                for s in range(lo, hi):
                    seg2g[s] = g

            def xchunk(s, c):
                g = seg2g[s]
                lo, hi = wbat[g]
                base = (c * (hi - lo) + (s - lo)) * SEG
                return gts[g][:, base:base + SEG]

            def wchunk(s, c):
                g = seg2g[s]
                lo, hi = wbat[g]
                base = 2 * (hi - lo) * SEG + (2 * (s - lo) + c) * F
                return gts[g][:, base:base + F]

            seg2o = {}
            for g, (lo, hi) in enumerate(obat):
                for s in range(lo, hi):
                    seg2o[s] = g
            yts = {}

            # All bias matmuls up front: they depend only on the tiny bconst
            # DMA (sem ~2us before the first stream group), so with one PSUM
            # bank per pair they fill the PE's otherwise-idle fill window and
            # come off the chunk-matmul steady state.  Each opens its pair's
            # accumulation group (start=True clears the bank's has_written
            # bits and seeds psum with the bias rows).
            pslist = []
            for plo, phi in pairs:
                pw = (phi - plo) * F
                ps = pspool.tile([P, 2 * F], f32, name="ps")
                pslist.append(ps)
                nc.tensor.matmul(
                    ps[:, 0:pw],
                    ob,
                    bc[:, P + plo * F:P + phi * F],
                    start=True,
                    stop=False,
                )

            for pi, (plo, phi) in enumerate(pairs):
                pw = (phi - plo) * F
                og = seg2o[plo]
                olo, ohi = obat[og]
                if og not in yts:
                    yts[og] = ypool.tile(
                        [P, (ohi - olo) * F], OUT_DT, tag=f"y{og}",
                        name=f"y{og}",
                    )
                yt = yts[og]
                ps = pslist[pi]
                for s in range(plo, phi):
                    o = (s - plo) * F
                    nc.tensor.matmul(
                        ps[:, o:o + F], xchunk(s, 0), wchunk(s, 0),
                        start=False, stop=False,
                    )
                    nc.tensor.matmul(
                        ps[:, o:o + F], xchunk(s, 1), wchunk(s, 1),
                        start=False, stop=(s == phi - 1),
                    )
                # relu on DVE: PSUM fp32 -> fp16 SBUF, no ACT table load.
                j = plo - olo
                nc.vector.tensor_scalar_max(
                    yt[:, j * F:j * F + pw], ps[:, 0:pw], 0.0
                )
                if phi == ohi:
                    # Early y groups ride the SWDGE FIFO behind all input
                    # groups so they can never preempt the input stream's
                    # HBM bandwidth; the last group uses the idle sync ring
                    # for the fastest possible final issue.
                    eng = nc.sync if og == len(obat) - 1 else nc.gpsimd
                    eng.dma_start(
                        y_d[:, olo:ohi, :].rearrange("p g f -> p (g f)"),
                        yt[:, 0:(ohi - olo) * F],
                    )
    nc.compile()
    return nc


def kernel(x: np.ndarray, idxs: np.ndarray, w: np.ndarray, b: np.ndarray) -> np.ndarray:
    global LAST_EXEC_TIME_NS, LAST_TRACE
    x = np.ascontiguousarray(x, dtype=np.float32)
    w = np.ascontiguousarray(w, dtype=np.float32)
    b = np.ascontiguousarray(b, dtype=np.float32)
    idxs_np = np.asarray(idxs).astype(np.int64)

    B = x.shape[0]
    order, per_core = _build_schedule(idxs_np)

    # Split each core's slot list into passes of <= MAX_NSEG slots.
    npass = max(1, (max(len(s) for s in per_core) + MAX_NSEG - 1) // MAX_NSEG)
    if npass == 1:
        nseg = max(2, max(len(s) for s in per_core))
    else:
        nseg = MAX_NSEG
    npad = nseg * SEG

    # Per-expert weight blocks in PE layout:
    # wprep[e, p, c*F + i] = w[e, i, c*P + p]  (c = contraction chunk 0/1)
    wprep = np.ascontiguousarray(
        w.transpose(0, 2, 1)           # (e, o, i)
        .reshape(64, 2, P, F)          # (e, c, p, i)
        .transpose(0, 2, 1, 3)         # (e, p, c, i)
        .reshape(64, P, 2 * F)
        .astype(MM_NP)
    )
    bprep = b.astype(MM_NP)

    nc = _build_program(nseg)
    trace = bool(os.environ.get("KBENCH_TRACE"))

    y = np.empty((B, F), dtype=np.float32)
    for pi in range(npass):
        in_maps = []
        for c in range(N_CORES):
            segs = per_core[c][pi * MAX_NSEG:(pi + 1) * MAX_NSEG]
            xpad = np.zeros((npad, F), dtype=MM_NP)
            eids = np.zeros(nseg, dtype=np.int64)
            for s, (e, g0, cnt) in enumerate(segs):
                xpad[s * SEG:s * SEG + cnt] = x[order[g0:g0 + cnt]]
                eids[s] = e
            # xt[p, c, n] = xpad[n, c*P + p]
            xt = xpad.T.reshape(2, P, npad).transpose(1, 0, 2)
            wseg = wprep[eids].transpose(1, 0, 2)  # (P, nseg, 2F)
            # packed per-group x+w stream tensor
            xw = np.concatenate(
                [
                    part
                    for lo, hi in _groups(nseg)
                    for part in (
                        xt[:, :, lo * SEG:hi * SEG].reshape(P, -1),
                        wseg[:, lo:hi, :].reshape(P, -1),
                    )
                ],
                axis=1,
            )
            bconst = np.concatenate(
                [np.ones(P, dtype=MM_NP), bprep[eids].reshape(nseg * F)]
            ).reshape(1, P + nseg * F)
            in_maps.append({"xw": xw, "bconst": bconst})

        res = run_bass_kernel_spmd(
            nc, in_maps, core_ids=list(range(N_CORES)), trace=trace
        )
        LAST_EXEC_TIME_NS = res.exec_time_ns
        LAST_TRACE = res.instructions_and_trace

        for c in range(N_CORES):
            segs = per_core[c][pi * MAX_NSEG:(pi + 1) * MAX_NSEG]
            ypad = (
                res.results[c]["y"]
                .transpose(1, 0, 2)
                .reshape(npad, F)
                .astype(np.float32)
            )
            for s, (e, g0, cnt) in enumerate(segs):
                y[order[g0:g0 + cnt]] = ypad[s * SEG:s * SEG + cnt]
    return y



# revision 13
# speedup vs baseline: 1.1028x; 1.1028x over previous
"""Trainium2 Bass kernel for batched per-sample expert matmul (MoE routing).

Computes y[n, i] = relu(b[idxs[n], i] + sum_o w[idxs[n], i, o] * x[n, o])
for x (8192, 256), idxs (8192,), w (64, 256, 256), b (64, 256).

Strategy (v2: expert-sharded, weight-stationary)
------------------------------------------------
Host side (numpy, cheap):
  * Group samples by expert; cut per-expert slots of <= 256 samples
    (one PSUM bank each); LPT-deal slots to the 8 cores balancing
    sample count.  Each expert's weight block is loaded ONCE per core
    (~1 MB/core vs 1.5 MB for slot-replicated loads), x/y ~0.5 MB each.
  * All matmul operands fp16 (absmax-rel err ~4e-4, gate is 2e-2).
  * Static SPMD program: slot widths W_s = max over cores of the
    rank-s slot's sample count (slots sorted desc), zero-padded.

Device side (one static Tile program on all 8 cores):
  Per slot s (expert e, W samples):  psum tile [128, 2W] (one bank)
      ps[i, n]      = sum_o wT[o, i]   * xT[o, n]      (4 matmuls:
        (o0,i0) start, (o1,i0), (o0,i1), (o1,i1) stop; weights are
        the stationary operand so PE cost scales with W, not 256)
      y[i', n]      = max(ps + b_col, 0)               (fused bias+relu,
        per-partition scalar; alternating DVE / ACT engines)
  Orientation note: output is yT (features on partitions); host
  transposes back.

  DMA plan: first slot group rides sync HWDGE (low first-byte latency);
  the bulk stream goes as few large packed x+w group DMAs on the gpsimd
  SWDGE queue (FIFO, one completion sem per group; issue cost ~0.65us
  each so groups hold >= 2 slots); bias consts ride scalar HWDGE.
  y outputs ride scalar HWDGE as their relus complete; the final small
  y group rides sync for the shortest last-receipt.

Host side: scatter yT columns back to sample order.
"""

import os

import numpy as np

import concourse.bacc as bacc
import concourse.bass as bass
import concourse.mybir as mybir
import concourse.tile as tile
from concourse.bass_utils import run_bass_kernel_spmd

N_CORES = 8
P = 128          # SBUF/PSUM partitions
F = 256          # feature dim (in == out == 256)
NEXP = 64
WMAX = 256       # samples per slot cap: psum tile [128, 2*W] fp32 <= one bank
MM_NP = np.float16
MM_DT = mybir.dt.float16
OUT_DT = mybir.dt.float16
F32 = mybir.dt.float32

# Set by the last kernel() call when KBENCH_TRACE=1 (used by test.py only).
LAST_EXEC_TIME_NS = None
LAST_TRACE = None


def _build_schedule(idxs: np.ndarray):
    """Sort samples by expert, cut <=WMAX-sample single-expert slots,
    LPT-deal slots to cores balancing total samples; sort each core's
    slots by count desc and compute static per-rank widths."""
    B = idxs.shape[0]
    order = np.argsort(idxs, kind="stable")
    sidx = idxs[order]
    slots = []  # (expert, global_start, count)
    i = 0
    while i < B:
        j = i
        while j < B and sidx[j] == sidx[i]:
            j += 1
        k = i
        while k < j:
            cnt = min(WMAX, j - k)
            slots.append((int(sidx[i]), k, cnt))
            k += cnt
        i = j
    # serpentine rank dealing: sort slots desc, rank r takes slots
    # [8r, 8r+8) (adjacent sizes -> minimal per-rank padding), serpentine
    # direction alternation balances per-core totals
    slots.sort(key=lambda t: -t[2])
    S = (len(slots) + N_CORES - 1) // N_CORES
    while len(slots) < S * N_CORES:
        slots.append((0, 0, 0))
    per_core = [[] for _ in range(N_CORES)]
    for r in range(S):
        rank = slots[r * N_CORES:(r + 1) * N_CORES]
        if r % 2 == 1:
            rank = rank[::-1]
        for c in range(N_CORES):
            per_core[c].append(rank[c])
    # static width per rank: max count across cores, multiple of 8, >= 8
    widths = []
    for s in range(S):
        w = max(pc[s][2] for pc in per_core)
        widths.append(max(8, (w + 7) // 8 * 8))
    return order, per_core, widths


def _in_groups(S):
    """Input stream slot groups: [1 (sync), 2s on gpsimd, single tail]."""
    out = [(0, 1)]
    lo = 1
    while lo < S - 1:
        hi = min(S - 1, lo + 2)
        out.append((lo, hi))
        lo = hi
    if S > 1:
        out.append((S - 1, S))
    return out


def _out_groups(S):
    """Output slot groups: pairs, except a single-slot tail group."""
    out = []
    lo = 0
    while lo < S - 1:
        hi = min(S - 1, lo + 2)
        out.append((lo, hi))
        lo = hi
    out.append((S - 1, S))
    return out


def _build_program(S, widths):
    nc = bacc.Bacc(
        "TRN2", target_bir_lowering=False, debug=False, num_devices=N_CORES
    )
    WSLOT = 4 * P  # w cols per slot (4 chunks of [128,128])
    cols = [WSLOT + 2 * w for w in widths]  # per-slot packed w+x cols
    igroups = _in_groups(S)
    ogroups = _out_groups(S)
    goff = []
    off = 0
    for lo, hi in igroups:
        goff.append(off)
        off += sum(cols[lo:hi])
    XWTOT = off
    yoff = []
    off = 0
    for s in range(S):
        yoff.append(off)
        off += 2 * widths[s]
    YTOT = off

    xw_d = nc.dram_tensor("xw", [P, XWTOT], MM_DT, kind="ExternalInput").ap()
    bc_d = nc.dram_tensor("bconst", [P, 2 * S], F32, kind="ExternalInput").ap()
    y_d = nc.dram_tensor("y", [P, YTOT], OUT_DT, kind="ExternalOutput").ap()

    with tile.TileContext(nc) as tc:
        with (
            tc.tile_pool(name="const", bufs=1) as const,
            tc.tile_pool(name="w", bufs=1) as wpool,
            tc.tile_pool(name="yout", bufs=1) as ypool,
            tc.tile_pool(name="ps", bufs=min(8, S), space="PSUM") as pspool,
        ):
            # bias columns on the scalar HWDGE ring (tiny, early); the dummy
            # activation hoists the one-time ACT table load (~2.7us) into the
            # stream shadow so the tail-slot activations run table-resident.
            # high_priority pins both at the front of the scalar queue.
            bc = const.tile([P, 2 * S], F32, tag="bconst")
            scratch = const.tile([1, 1], F32, tag="actwarm")
            with tc.high_priority():
                nc.scalar.dma_start(bc[:], bc_d[:])
                nc.scalar.activation(
                    scratch[:], bc[0:1, 0:1],
                    mybir.ActivationFunctionType.Relu,
                )

            # all input groups ride the sync HWDGE ring: strict FIFO delivery,
            # no SWDGE Q7 emission stalls, and ~0.5us faster completion
            # receipts than SWDGE (the receipt gates each slot's matmuls)
            gts = {}
            for g, (lo, hi) in enumerate(igroups):
                gw = sum(cols[lo:hi])
                t = wpool.tile([P, gw], MM_DT, tag=f"xw{g}", name=f"xw{g}")
                gts[g] = t
                if g == len(igroups) - 1 and hi - lo == 1:
                    # split last group into w-part and x-part for an earlier
                    # completion sem on the critical tail
                    wpart = 4 * P
                    nc.sync.dma_start(
                        t[:, 0:wpart], xw_d[:, goff[g]:goff[g] + wpart]
                    )
                    nc.sync.dma_start(
                        t[:, wpart:gw],
                        xw_d[:, goff[g] + wpart:goff[g] + gw],
                    )
                else:
                    nc.sync.dma_start(t[:], xw_d[:, goff[g]:goff[g] + gw])

            seg2g = {}
            for g, (lo, hi) in enumerate(igroups):
                for s in range(lo, hi):
                    seg2g[s] = g

            def wchunk(s, c):
                g = seg2g[s]
                lo, hi = igroups[g]
                base = sum(cols[lo:s]) + c * P
                return gts[g][:, base:base + P]

            def xchunk(s, c):
                g = seg2g[s]
                lo, hi = igroups[g]
                w = widths[s]
                base = sum(cols[lo:s]) + WSLOT + c * w
                return gts[g][:, base:base + w]

            seg2o = {}
            for g, (lo, hi) in enumerate(ogroups):
                for s in range(lo, hi):
                    seg2o[s] = g
            yts = {}

            for s in range(S):
                w = widths[s]
                ps = pspool.tile([P, 512], F32, name="ps")
                # 4 matmuls: (o0,i0) start, (o1,i0), (o0,i1), (o1,i1) stop
                nc.tensor.matmul(
                    ps[:, 0:w], wchunk(s, 0), xchunk(s, 0),
                    start=True, stop=False,
                )
                nc.tensor.matmul(
                    ps[:, 0:w], wchunk(s, 1), xchunk(s, 1),
                    start=False, stop=False,
                )
                nc.tensor.matmul(
                    ps[:, w:2 * w], wchunk(s, 2), xchunk(s, 0),
                    start=False, stop=False,
                )
                nc.tensor.matmul(
                    ps[:, w:2 * w], wchunk(s, 3), xchunk(s, 1),
                    start=False, stop=True,
                )

                og = seg2o[s]
                olo, ohi = ogroups[og]
                if og not in yts:
                    gw = sum(2 * widths[t_] for t_ in range(olo, ohi))
                    yts[og] = ypool.tile(
                        [P, gw], OUT_DT, tag=f"y{og}", name=f"y{og}"
                    )
                yt = yts[og]
                j = yoff[s] - yoff[olo]
                # fused bias + relu, per-partition bias scalar.  DVE is the
                # busiest engine mid-kernel, so tail slots split their two
                # chunk-relus across DVE and the (table-resident) ACT engine
                # to halve the post-last-matmul relu wall time.
                nc.vector.tensor_scalar(
                    yt[:, j:j + w], ps[:, 0:w],
                    bc[:, 2 * s:2 * s + 1], 0.0,
                    mybir.AluOpType.add, mybir.AluOpType.max,
                )
                if s >= S - 3:
                    nc.scalar.activation(
                        yt[:, j + w:j + 2 * w], ps[:, w:2 * w],
                        mybir.ActivationFunctionType.Relu,
                        bias=bc[:, 2 * s + 1:2 * s + 2],
                    )
                else:
                    nc.vector.tensor_scalar(
                        yt[:, j + w:j + 2 * w], ps[:, w:2 * w],
                        bc[:, 2 * s + 1:2 * s + 2], 0.0,
                        mybir.AluOpType.add, mybir.AluOpType.max,
                    )
                if s == ohi - 1:
                    gw = sum(2 * widths[t_] for t_ in range(olo, ohi))
                    # early y groups ride the otherwise-idle SWDGE queue;
                    # the next-to-last pair rides sync behind the inputs;
                    # the final (smallest) group rides scalar so its issue
                    # starts the moment its relus land
                    if og == len(ogroups) - 1:
                        oeng = nc.scalar
                    elif og >= len(ogroups) - 3:
                        oeng = nc.sync
                    else:
                        oeng = nc.gpsimd
                    oeng.dma_start(
                        y_d[:, yoff[olo]:yoff[olo] + gw], yt[:, 0:gw]
                    )
    nc.compile()
    return nc


def kernel(x: np.ndarray, idxs: np.ndarray, w: np.ndarray, b: np.ndarray) -> np.ndarray:
    global LAST_EXEC_TIME_NS, LAST_TRACE
    x = np.ascontiguousarray(x, dtype=np.float32)
    w = np.ascontiguousarray(w, dtype=np.float32)
    b = np.ascontiguousarray(b, dtype=np.float32)
    idxs_np = np.asarray(idxs).astype(np.int64)

    B = x.shape[0]
    order, per_core, widths = _build_schedule(idxs_np)
    S = len(widths)

    # per-expert weights in PE layout: [o, i] chunks packed [128, 512]
    # cols: (o0,i0)(o1,i0)(o0,i1)(o1,i1)
    wT = w.transpose(0, 2, 1).astype(MM_NP)  # (e, o, i)
    wprep = np.concatenate(
        [
            wT[:, 0:P, 0:P], wT[:, P:F, 0:P],
            wT[:, 0:P, P:F], wT[:, P:F, P:F],
        ],
        axis=2,
    )  # (e, 128, 512)

    xT = x.T.astype(MM_NP)  # (256, B)

    igroups = _in_groups(S)
    WSLOT = 4 * P

    nc = _build_program(S, widths)
    trace = bool(os.environ.get("KBENCH_TRACE"))

    in_maps = []
    for c in range(N_CORES):
        parts = []
        for lo, hi in igroups:
            for s in range(lo, hi):
                e, g0, cnt = per_core[c][s]
                ws = widths[s]
                parts.append(wprep[e])
                xs = np.zeros((2 * P, ws), dtype=MM_NP)
                if cnt:
                    xs[:, :cnt] = xT[:, order[g0:g0 + cnt]]
                parts.append(xs.reshape(2, P, ws).transpose(1, 0, 2).reshape(P, 2 * ws))
        xw = np.ascontiguousarray(np.concatenate(parts, axis=1))
        bcd = np.zeros((P, 2 * S), dtype=np.float32)
        for s in range(S):
            e = per_core[c][s][0]
            bcd[:, 2 * s] = b[e, 0:P]
            bcd[:, 2 * s + 1] = b[e, P:F]
        in_maps.append({"xw": xw, "bconst": bcd})

    res = run_bass_kernel_spmd(
        nc, in_maps, core_ids=list(range(N_CORES)), trace=trace
    )
    LAST_EXEC_TIME_NS = res.exec_time_ns
    LAST_TRACE = res.instructions_and_trace

    y = np.empty((B, F), dtype=np.float32)
    yoff = np.cumsum([0] + [2 * w_ for w_ in widths])
    for c in range(N_CORES):
        yT = res.results[c]["y"].astype(np.float32)  # (128, YTOT)
        for s in range(S):
            e, g0, cnt = per_core[c][s]
            if not cnt:
                continue
            sl = order[g0:g0 + cnt]
            o = yoff[s]
            ws = widths[s]
            y[sl, 0:P] = yT[:, o:o + cnt].T
            y[sl, P:F] = yT[:, o + ws:o + ws + cnt].T
    return y
